# revision 23
# baseline (speedup 1.0000x reference)
"""Trainium2 Bass kernel for PVT-style spatial-reduction attention (v3).

Shapes (hardcoded): x [2, 4096, 256], HEAD=8, dh=32, SR=2, R=8, H=W=64.
Sharding: core c = (batch b = c//4, query block j = c%4). Each core computes
q/attention/proj for its 1024 query rows and redundantly computes the small
conv+LN+KV path for its batch. Per-core x is pre-rotated on host so each
core's own query block is rows 0:1024.

v3 design:
- fp16 compute chain (conv/kv/q/pv/proj); fp8e4m3 DoubleRow score matmuls
  (0.5 cyc/row) via host-permuted q/k projection columns + per-head repack
  DMAs to 32-aligned lane bases.
- Transposed pv (pts stationary): 33-row pv matmuls, per-partition softmax
  denominators (single reciprocal + stride-0 scale, no broadcasts).
- V computed token-major directly (xsb stationary) - no PE transposes.
- Softmax exp split across ScalarE (true Exp) and DVE (one-op Schraudolph
  int16 bit-trick into fp16); per-head schedule interleaves the engines.
- Attention starts after strip 0: scores for kv blocks 0-3 of all heads are
  emitted interleaved with strip-1 conv/kv so ScalarE/DVE start ~15us in.
- Few large DMAs; conv weights in a separate first blob for an early start.

PSUM (8 banks): P1 2x[128,1024]f32 (4) for q-path/scores/proj, CVKV
2x[128,512]f32 (2) for conv/kv/v/transposes, PV 2x[128,8,33]f32 (2) for pv.
"""
import sys

if "/opt/trn_rl_repo" not in sys.path:
    sys.path.insert(0, "/opt/trn_rl_repo")

import numpy as np

F16NP = np.float16

HEAD, DH, C, N, B, M, R = 8, 32, 256, 4096, 2, 1024, 8
NB = N // 4          # query rows per core
SCALE = DH ** -0.5
NCORES = 8
MAGIC = 0x5F3759DF
LOG2E8 = 1477.3195879  # 2^10 / ln 2
BITB = 15317.95        # tuned fp16 Schraudolph offset (trunc semantics)

# per-head exp engine schedule over mc=0..7 (A=ScalarE exp, D=DVE bit trick);
# alternating 6A2D / 5A3D heads -> 44 ACT / 20 DVE tiles.
EXP_SCHED = ["AAAADDAA"] * 5 + ["AAAADDAD"] * 1 + ["AAAADADA"] * 2

# conv blob (wbc): srwT 2048 | ones1c 1
OC_SRWT, OC_ONE1C = 0, 2048
WBC_COLS = 2064
# main blob (wb) column offsets (fp16 elements)
O_QWT, O_KVWT, O_PWT = 0, 512, 1536
O_AQT, O_AVT, O_IDENT, O_BQT = 2048, 2064, 2080, 2208
O_BVTK, O_BVTV, O_WG1K, O_WG1V = 2464, 2720, 2976, 3232
O_AVG1, O_ONESR, O_PBROW = 3488, 3496, 3624
BLOB_COLS = 3880

_CACHE = {}
DEBUG = False


def _build_program():
    import concourse.bass as bass
    import concourse.tile as tile
    from concourse.bacc import Bacc
    from concourse import mybir

    F32 = mybir.dt.float32
    F16 = mybir.dt.float16
    FP8 = mybir.dt.float8e4
    I16 = mybir.dt.int16
    I32 = mybir.dt.int32
    U8 = mybir.dt.uint8
    AF = mybir.ActivationFunctionType
    ALU = mybir.AluOpType
    PM = mybir.MatmulPerfMode

    nc = Bacc()
    P = 128
    ST = 512       # kv tokens per strip

    def s0(ap, n):
        # stride-0 broadcast along a new innermost free dim
        return bass.AP(tensor=ap.tensor, offset=ap.offset,
                       ap=[list(d) for d in ap.ap] + [[0, n]])

    def bcast(ap, nparts):
        return bass.AP(tensor=ap.tensor, offset=ap.offset,
                       ap=[[0, nparts]] + [list(d) for d in ap.ap])

    xT_d = nc.declare_dram_parameter("xT", [P, 2, N], F16, isOutput=False)
    wbc_d = nc.declare_dram_parameter("wbc", [P, WBC_COLS], F16, isOutput=False)
    wb_d = nc.declare_dram_parameter("wb", [P, BLOB_COLS], F16, isOutput=False)
    wf_d = nc.declare_dram_parameter("wf", [P, 4], F32, isOutput=False)
    pb_d = nc.declare_dram_parameter("pb", [C], F32, isOutput=False)
    out_d = nc.declare_dram_parameter("out", [NB, C], F32, isOutput=True)
    if DEBUG:
        dbg = {
            "d_xsb": nc.declare_dram_parameter("d_xsb", [2, P, 2, 512], F16, isOutput=True),
            "d_an": nc.declare_dram_parameter("d_an", [2, P, 4], F32, isOutput=True),
            "d_k8": nc.declare_dram_parameter("d_k8", [P, 2, M], U8, isOutput=True),
            "d_q8": nc.declare_dram_parameter("d_q8", [P, 2, NB], U8, isOutput=True),
            "d_kA": nc.declare_dram_parameter("d_kA", [P, 2, M], U8, isOutput=True),
            "d_qA": nc.declare_dram_parameter("d_qA", [P, 2, NB], U8, isOutput=True),
            "d_vsb": nc.declare_dram_parameter("d_vsb", [2, P, 4, HEAD, DH + 1], F16, isOutput=True),
            "d_pts0": nc.declare_dram_parameter("d_pts0", [P, 8, NB], F16, isOutput=True),
            "d_rec": nc.declare_dram_parameter("d_rec", [HEAD, P, 8], F32, isOutput=True),
            "d_onorm": nc.declare_dram_parameter("d_onorm", [P, 8, HEAD, DH], F16, isOutput=True),
            "d_outT": nc.declare_dram_parameter("d_outT", [P, 2, NB], F16, isOutput=True),
        }

    with tile.TileContext(nc) as tc:
        with tc.tile_pool(name="wgt", bufs=1) as WGT, \
             tc.tile_pool(name="acts", bufs=1) as ACTS, \
             tc.tile_pool(name="str", bufs=2) as STR, \
             tc.tile_pool(name="tmp", bufs=2) as TMP, \
             tc.tile_pool(name="pts", bufs=8) as PTS, \
             tc.tile_pool(name="fin", bufs=8) as FIN, \
             tc.tile_pool(name="p1", bufs=3, space="PSUM") as P1, \
             tc.tile_pool(name="cvkv", bufs=2, space="PSUM") as CVKV, \
             tc.tile_pool(name="dscr", bufs=2, space="DRAM") as DSCR:

            # ---------------- loads (order matters for early start) ----------
            wbc = WGT.tile([P, WBC_COLS], F16, tag="wbc")
            nc.sync.dma_start(out=wbc[:], in_=wbc_d[:])
            wf = WGT.tile([P, 4], F32, tag="wf")
            nc.sync.dma_start(out=wf[:], in_=wf_d[:])
            xs0 = ACTS.tile([P, 2, 2048], F16, tag="xT0")
            nc.sync.dma_start(out=xs0[:], in_=xT_d[:, :, 0:2048])
            wb = WGT.tile([P, BLOB_COLS], F16, tag="wb")
            nc.sync.dma_start(out=wb[:], in_=wb_d[:])
            xs1 = ACTS.tile([P, 2, 2048], F16, tag="xT1")
            nc.sync.dma_start(out=xs1[:], in_=xT_d[:, :, 2048:4096])
            pbB = WGT.tile([P, C], F32, tag="pbB")
            nc.sync.dma_start(out=pbB[:], in_=bcast(pb_d.ap(), P))
            xTs = [xs0, xs1]

            srwT = wbc[:, OC_SRWT:OC_SRWT + 2048].rearrange(
                "p (ch t c) -> p ch t c", ch=2, t=4)
            ones1c = wbc[:, OC_ONE1C:OC_ONE1C + 1]
            qwT = wb[:, O_QWT:O_QWT + 512].rearrange("p (ch c) -> p ch c", ch=2)
            kvwT = wb[:, O_KVWT:O_KVWT + 1024].rearrange("p (ch c) -> p ch c", ch=2)
            pwT = wb[:, O_PWT:O_PWT + 512].rearrange("p (ch c) -> p ch c", ch=2)
            aqT = wb[:, O_AQT:O_AQT + 16].rearrange("p (ch r) -> p ch r", ch=2)
            avT = wb[:, O_AVT:O_AVT + 16].rearrange("p (ch r) -> p ch r", ch=2)
            ident = wb[:, O_IDENT:O_IDENT + 128]
            bqT = wb[0:R, O_BQT:O_BQT + 256].rearrange("r (ch c) -> r ch c", ch=2)
            bvTk = wb[0:R, O_BVTK:O_BVTK + 256].rearrange("r (ch c) -> r ch c", ch=2)
            bvTv = wb[0:R, O_BVTV:O_BVTV + 256]
            wg1k = wb[0:1, O_WG1K:O_WG1K + 256].rearrange("a (ch c) -> a ch c", ch=2)
            wg1v = wb[0:1, O_WG1V:O_WG1V + 256]
            avg1 = wb[0:1, O_AVG1:O_AVG1 + 8]
            onesr = wb[0:1, O_ONESR:O_ONESR + 128]
            pbrow = wb[0:1, O_PBROW:O_PBROW + 256]

            # persistent activations
            k8tmp = ACTS.tile([P, 2, M], FP8, tag="k8tmp")
            q8tmp = ACTS.tile([P, 2, NB], FP8, tag="q8tmp")
            kO = ACTS.tile([P, 2, M], FP8, tag="kO")
            qO = ACTS.tile([P, 2, NB], FP8, tag="qO")
            outT = ACTS.tile([P, 2, NB], F16, tag="outT")
            onorm = ACTS.tile([P, 8, HEAD, DH], F16, tag="onorm")
            tq = ACTS.tile([R, NB], F16, tag="tq")

            xsbs, sqs, negmus, t2s, vsbs, ans, ascls, abits = \
                [], [], [], [], [], [], [], []

            # ---------------- per-strip pieces as emission chunks ------------
            def conv_oc(s, oc):
                xs_t = xTs[s]
                if oc == 0:
                    xsb_s = STR.tile([P, 2, ST], F16, tag="xsb")
                    xsbs.append(xsb_s)
                xsb_s = xsbs[s]
                cps = CVKV.tile([P, ST], F32, tag="cv")
                first = True
                for cc in range(2):
                    xv = xs_t[:, cc, :].rearrange(
                        "p (i a j b) -> p i a j b", i=16, a=2, j=32, b=2)
                    for di in range(2):
                        for dj in range(2):
                            nc.tensor.matmul(
                                cps[:], srwT[:, cc, di * 2 + dj,
                                             oc * P:(oc + 1) * P],
                                xv[:, :, di, :, dj],
                                start=first,
                                stop=(cc == 1 and di == 1 and dj == 1))
                            first = False
                if s == 0:
                    nc.scalar.activation(out=xsb_s[:, oc, :], in_=cps[:],
                                         func=AF.Identity,
                                         bias=wf[:, 2 + oc:3 + oc])
                else:
                    nc.vector.tensor_scalar_add(
                        out=xsb_s[:, oc, :], in0=cps[:], scalar1=wf[:, 2 + oc:3 + oc])
                if DEBUG and oc == 1:
                    nc.sync.dma_start(out=dbg["d_xsb"][s], in_=xsb_s[:])

            def stats(s):
                xsb_s = xsbs[s]
                sq_s = STR.tile([P, 2, ST], F16, tag="sq")
                nc.gpsimd.tensor_mul(out=sq_s[:], in0=xsb_s[:], in1=xsb_s[:])
                sxp = CVKV.tile([1, ST], F32, tag="cv")
                nc.tensor.matmul(sxp[:], ones1c, xsb_s[:, 0, :], start=True, stop=False)
                nc.tensor.matmul(sxp[:], ones1c, xsb_s[:, 1, :], start=False, stop=True)
                negmu = TMP.tile([1, ST], F16, tag="negmu")
                nc.vector.tensor_scalar_mul(out=negmu[:], in0=sxp[:], scalar1=-1.0)
                negmus.append(negmu)
                sxxp = CVKV.tile([1, ST], F32, tag="cv")
                nc.tensor.matmul(sxxp[:], ones1c, sq_s[:, 0, :], start=True, stop=False)
                nc.tensor.matmul(sxxp[:], ones1c, sq_s[:, 1, :], start=False, stop=True)
                ex2_sb = TMP.tile([1, ST], F32, tag="ex2sb")
                nc.vector.tensor_copy(out=ex2_sb[:], in_=sxxp[:])

                # chunk-major repack [1,512] -> [128,4] via DRAM bounce
                nm_d = DSCR.tile([ST], F16, tag=f"nm{s}")
                nc.sync.dma_start(out=nm_d[:], in_=negmu[:])
                ex_d = DSCR.tile([ST], F32, tag=f"ex{s}")
                nc.sync.dma_start(out=ex_d[:], in_=ex2_sb[:])
                mur = TMP.tile([P, 4], F16, tag="mur")
                nc.sync.dma_start(out=mur[:],
                                  in_=nm_d[:].rearrange("(g p) -> p g", p=P))
                ex2r = TMP.tile([P, 4], F32, tag="ex2r")
                nc.sync.dma_start(out=ex2r[:],
                                  in_=ex_d[:].rearrange("(g p) -> p g", p=P))

                # rstd via quake rsqrt (1 newton), [128,4] chain on DVE
                nmu2 = TMP.tile([P, 4], F32, tag="nmu2")
                nc.vector.scalar_tensor_tensor(out=nmu2[:], in0=mur[:], scalar=-1.0,
                                               in1=mur[:], op0=ALU.mult, op1=ALU.mult)
                ve = TMP.tile([P, 4], F32, tag="ve")
                nc.vector.scalar_tensor_tensor(out=ve[:], in0=nmu2[:], scalar=1e-5,
                                               in1=ex2r[:], op0=ALU.add, op1=ALU.add)
                hsh = TMP.tile([P, 4], I32, tag="hsh")
                nc.vector.tensor_scalar(out=hsh[:], in0=ve[:].bitcast(I32), scalar1=1,
                                        scalar2=None, op0=ALU.logical_shift_right)
                nc.vector.tensor_scalar(out=hsh[:], in0=hsh[:], scalar1=-1,
                                        scalar2=MAGIC, op0=ALU.mult, op1=ALU.add)
                y0 = hsh[:].bitcast(F32)
                nt = TMP.tile([P, 4], F32, tag="nt")
                nc.vector.tensor_mul(out=nt[:], in0=y0, in1=y0)
                nc.vector.scalar_tensor_tensor(out=nt[:], in0=nt[:], scalar=-0.5,
                                               in1=ve[:], op0=ALU.mult, op1=ALU.mult)
                nc.vector.tensor_scalar_add(out=nt[:], in0=nt[:], scalar1=1.5)
                an_s = STR.tile([P, 4], F32, tag="an")
                nc.vector.tensor_mul(out=an_s[:], in0=y0, in1=nt[:])
                ascl_s = STR.tile([P, 4], F32, tag="ascl")
                nc.vector.tensor_scalar_mul(out=ascl_s[:], in0=an_s[:], scalar1=SCALE)
                abit_s = STR.tile([P, 4], F32, tag="abit")
                nc.vector.tensor_scalar_mul(out=abit_s[:], in0=ascl_s[:],
                                            scalar1=LOG2E8)
                ans.append(an_s)
                ascls.append(ascl_s)
                abits.append(abit_s)
                if DEBUG:
                    nc.sync.dma_start(out=dbg["d_an"][s], in_=an_s[:])

            def t2_and_k(s):
                xsb_s, negmu = xsbs[s], negmus[s]
                t2p = CVKV.tile([R, ST], F32, tag="cv")
                nc.tensor.matmul(t2p[:], avT[:, 0, :], xsb_s[:, 0, :], start=True, stop=False)
                nc.tensor.matmul(t2p[:], avT[:, 1, :], xsb_s[:, 1, :], start=False, stop=False)
                nc.tensor.matmul(t2p[:], avg1, negmu[:], start=False, stop=True)
                t2 = TMP.tile([R, ST], F16, tag="t2")
                nc.vector.tensor_copy(out=t2[:], in_=t2p[:])
                t2s.append(t2)
                for kvoc in range(2):
                    kps = CVKV.tile([P, ST], F32, tag="cv")
                    nc.tensor.matmul(kps[:], kvwT[:, 0, kvoc * P:(kvoc + 1) * P],
                                     xsb_s[:, 0, :], start=True, stop=False)
                    nc.tensor.matmul(kps[:], kvwT[:, 1, kvoc * P:(kvoc + 1) * P],
                                     xsb_s[:, 1, :], start=False, stop=False)
                    nc.tensor.matmul(kps[:], wg1k[:, kvoc, :], negmu[:],
                                     start=False, stop=False)
                    nc.tensor.matmul(kps[:], bvTk[:, kvoc, :], t2[:],
                                     start=False, stop=True)
                    if s == 0:
                        nc.scalar.copy(out=k8tmp[:, kvoc, s * ST:(s + 1) * ST],
                                       in_=kps[:])
                    else:
                        nc.vector.tensor_copy(
                            out=k8tmp[:, kvoc, s * ST:(s + 1) * ST], in_=kps[:])

            def k_repack(s):
                # odd heads h=2a+1 (packed base 16h=32a+16) -> kO base 32a
                for a in range(4):
                    h = 2 * a + 1
                    nc.gpsimd.dma_start(
                        out=kO[32 * a:32 * a + 16, :, s * ST:(s + 1) * ST],
                        in_=k8tmp[16 * h:16 * h + 16, :, s * ST:(s + 1) * ST])

            def v_path(s):
                # token-major v: out[m, vchan] via xsb/negmu/t2 as stationary
                xsb_s, negmu, t2, an_s = xsbs[s], negmus[s], t2s[s], ans[s]
                vsb_s = STR.tile([P, 4, HEAD, DH + 1], F16, tag="vsb")
                for ml in range(4):
                    sl = slice(ml * P, (ml + 1) * P)
                    vP = CVKV.tile([P, C], F32, tag="cv")
                    nc.tensor.matmul(vP[:], xsb_s[:, 0, sl], kvwT[:, 0, 256:512],
                                     start=True, stop=False)
                    nc.tensor.matmul(vP[:], xsb_s[:, 1, sl], kvwT[:, 1, 256:512],
                                     start=False, stop=False)
                    nc.tensor.matmul(vP[:], negmu[:, sl], wg1v,
                                     start=False, stop=False, tile_position=(0, 0))
                    nc.tensor.matmul(vP[:], t2[:, sl], bvTv,
                                     start=False, stop=True, tile_position=(0, 0))
                    nc.vector.tensor_scalar_mul(
                        out=vsb_s[:, ml, :, 0:DH],
                        in0=vP[:].rearrange("p (h d) -> p h d", d=DH),
                        scalar1=an_s[:, ml:ml + 1])
                nc.gpsimd.memset(vsb_s[:, :, :, DH:DH + 1], 1.0)
                vsbs.append(vsb_s)
                if DEBUG:
                    nc.sync.dma_start(out=dbg["d_vsb"][s], in_=vsb_s[:])

            def q_path():
                xs_t = xTs[0]
                tqp = P1.tile([R, NB], F32, tag="p1")
                for nh in range(2):
                    sl = slice(nh * 512, (nh + 1) * 512)
                    nc.tensor.matmul(tqp[:, sl], aqT[:, 0, :], xs_t[:, 0, sl],
                                     start=True, stop=False)
                    nc.tensor.matmul(tqp[:, sl], aqT[:, 1, :], xs_t[:, 1, sl],
                                     start=False, stop=True)
                nc.vector.tensor_copy(out=tq[:], in_=tqp[:])
                for oc in range(2):
                    qps = P1.tile([P, NB], F32, tag="p1")
                    for nh in range(2):
                        sl = slice(nh * 512, (nh + 1) * 512)
                        nc.tensor.matmul(qps[:, sl],
                                         qwT[:, 0, oc * P:(oc + 1) * P],
                                         xs_t[:, 0, sl], start=True, stop=False)
                        nc.tensor.matmul(qps[:, sl],
                                         qwT[:, 1, oc * P:(oc + 1) * P],
                                         xs_t[:, 1, sl], start=False, stop=False)
                        nc.tensor.matmul(qps[:, sl], bqT[:, oc, :], tq[:, sl],
                                         start=False, stop=True)
                    nc.vector.tensor_scalar_add(
                        out=q8tmp[:, oc, :], in0=qps[:], scalar1=wf[:, oc:oc + 1])
                for a in range(4):
                    h = 2 * a + 1
                    nc.sync.dma_start(
                        out=qO[32 * a:32 * a + 16],
                        in_=q8tmp[16 * h:16 * h + 16])

            # ---------------- attention pieces ----------------
            ptshs = [None] * HEAD

            def score_pair(h, mcp):
                # scores+exp for mc = 2*mcp, 2*mcp+1; even heads read the
                # packed tiles directly (base 16h is 32-aligned), odd heads
                # the repacked kO/qO
                if h % 2 == 0:
                    a, kT, qT8 = h // 2, k8tmp, q8tmp
                else:
                    a, kT, qT8 = h // 2, kO, qO
                if ptshs[h] is None:
                    ptsh = PTS.tile([P, 8, NB], F16, tag="ptsh")
                    ptshs[h] = ptsh
                ptsh = ptshs[h]
                for mc in (2 * mcp, 2 * mcp + 1):
                    s_, ml = mc // 4, mc % 4
                    stile = P1.tile([P, NB], F32, tag="p1")
                    for qc in range(4):
                        nc.tensor.matmul(
                            stile[:, qc * 256:(qc + 1) * 256],
                            kT[32 * a:32 * a + 16, :, mc * P:(mc + 1) * P],
                            qT8[32 * a:32 * a + 16, :, qc * 256:(qc + 1) * 256],
                            start=True, stop=True, perf_mode=PM.DoubleRow,
                            tile_position=(32 * a, 0))
                    if EXP_SCHED[h][mc] == "A":
                        nc.scalar.activation(out=ptsh[:, mc, :], in_=stile[:],
                                             func=AF.Exp,
                                             scale=ascls[s_][:, ml:ml + 1])
                    else:
                        nc.vector.tensor_scalar(
                            out=ptsh[:, mc, :].bitcast(I16), in0=stile[:],
                            scalar1=abits[s_][:, ml:ml + 1], scalar2=BITB,
                            op0=ALU.mult, op1=ALU.add)

            def pv_head(h):
                ptsh = ptshs[h]
                pvq = CVKV.tile([P, 8, DH + 1], F32, tag="cv")
                for qb in range(8):
                    for mc in range(8):
                        nc.tensor.matmul(
                            pvq[:, qb, :],
                            ptsh[:, mc, qb * P:(qb + 1) * P],
                            vsbs[mc // 4][:, mc % 4, h, :],
                            start=(mc == 0), stop=(mc == 7),
                            tile_position=(0, 0))
                rec = TMP.tile([P, 8], F32, tag="rec")
                nc.vector.reciprocal(out=rec[:], in_=pvq[:, :, DH])
                nc.vector.tensor_tensor(out=onorm[:, :, h, :],
                                        in0=pvq[:, :, 0:DH],
                                        in1=s0(rec[:], DH), op=ALU.mult)
                if DEBUG:
                    if h == 0:
                        nc.sync.dma_start(out=dbg["d_pts0"][:], in_=ptsh[:])
                    nc.sync.dma_start(out=dbg["d_rec"][h], in_=rec[:])

            def transpose_ch(ch):
                for qb in range(8):
                    ttile = CVKV.tile([P, P], F16, tag="cv")
                    nc.tensor.transpose(
                        ttile[:],
                        onorm[:, qb, ch * 4:(ch + 1) * 4, :].rearrange(
                            "p h d -> p (h d)"),
                        ident)
                    if ch == 1 and qb % 2 == 0:
                        nc.scalar.copy(out=outT[:, ch, qb * P:(qb + 1) * P],
                                       in_=ttile[:])
                    else:
                        nc.vector.tensor_copy(
                            out=outT[:, ch, qb * P:(qb + 1) * P], in_=ttile[:])

            # ---------------- emission schedule ----------------
            conv_oc(0, 0)
            conv_oc(0, 1)
            stats(0)
            t2_and_k(0)
            k_repack(0)
            q_path()
            v_path(0)

            # interleave strip 1 with early scores (all heads, kv blocks 0-3)
            s1_chunks = [
                lambda: conv_oc(1, 0),
                lambda: conv_oc(1, 1),
                lambda: stats(1),
                lambda: t2_and_k(1),
                lambda: (k_repack(1), v_path(1)),
            ]
            b1_chunks = [(h, mcp) for h in (0, 2, 4, 6, 1, 3, 5, 7)
                         for mcp in (0, 1)]
            bi = 0
            for chunk in s1_chunks:
                chunk()
                for _ in range(2):
                    if bi < len(b1_chunks):
                        score_pair(*b1_chunks[bi])
                        bi += 1
            while bi < len(b1_chunks):
                score_pair(*b1_chunks[bi])
                bi += 1

            # second half: kv blocks 4-7 per head, pv pipelined one head behind
            for h in range(HEAD):
                score_pair(h, 2)
                score_pair(h, 3)
                if h > 0:
                    pv_head(h - 1)
                if h == 4:
                    transpose_ch(0)   # heads 0-3 normalized by now
            pv_head(HEAD - 1)
            if DEBUG:
                nc.sync.dma_start(out=dbg["d_onorm"][:], in_=onorm[:])
            transpose_ch(1)
            if DEBUG:
                nc.sync.dma_start(out=dbg["d_outT"][:], in_=outT[:])

            # ---------------- projection (per-t8, pipelined) ----------------
            for t8 in range(8):
                pp = P1.tile([P, C], F32, tag="p1")
                nc.tensor.matmul(pp[:], outT[:, 0, t8 * P:(t8 + 1) * P],
                                 pwT[:, 0, :], start=True, stop=False)
                nc.tensor.matmul(pp[:], outT[:, 1, t8 * P:(t8 + 1) * P],
                                 pwT[:, 1, :], start=False, stop=False)
                nc.tensor.matmul(pp[:], onesr, pbrow,
                                 start=False, stop=True, tile_position=(0, 0))
                fin = FIN.tile([P, C], F32, tag="fin")
                if t8 % 2 == 0:
                    nc.scalar.copy(out=fin[:], in_=pp[:])
                else:
                    nc.vector.tensor_copy(out=fin[:], in_=pp[:])
                nc.sync.dma_start(out=out_d[t8 * P:(t8 + 1) * P], in_=fin[:])

    nc.finalize()
    return nc


P_ = 128


def _kq_perm():
    # PSUM chunk oc holds half `oc` of all heads: partition p=16h+lane ->
    # original channel 32h + 16*oc + lane
    perm = np.empty(2 * P_, np.int64)
    for oc in range(2):
        for p in range(P_):
            perm[oc * P_ + p] = 32 * (p // 16) + 16 * oc + (p % 16)
    return perm


def _prep_shared(q_w, q_b, kv_w, kv_b, proj_w, proj_b, a_q, b_q, a_v, b_v,
                 sr_w, sr_b, ln_g, ln_b):
    f32 = np.float32

    def chunkT(w):  # [out, in] -> [128, n_in_chunks, out]
        wt = np.ascontiguousarray(np.asarray(w, f32).T)
        ic, oc = wt.shape
        return np.ascontiguousarray(
            wt.reshape(ic // 128, 128, oc).transpose(1, 0, 2)).astype(F16NP)

    kv_w = np.asarray(kv_w, f32)
    a_v = np.asarray(a_v, f32)
    b_v = np.asarray(b_v, f32)
    b_q = np.asarray(b_q, f32)
    q_w = np.asarray(q_w, f32)
    q_b = np.asarray(q_b, f32)
    g = np.asarray(ln_g, f32)
    bb = np.asarray(ln_b, f32)
    proj_w = np.asarray(proj_w, f32)

    # fold LayerNorm gamma into kv/a_v weights; mean via rank-1 correction;
    # k-side constants dropped (softmax shift invariance), v-side constants
    # folded into the projection bias.
    Wg = kv_w * g[None, :]
    wg1 = Wg.sum(1)
    Avg = a_v * g[None, :]
    avg1 = Avg.sum(1)
    wbt = kv_w @ bb + np.asarray(kv_b, f32)
    dconst = b_v @ (a_v @ bb)
    wv_const = wbt[C:] + dconst
    pb_eff = np.asarray(proj_b, f32) + proj_w @ wv_const

    perm = _kq_perm()
    qw_p = q_w[perm]
    qb_p = q_b[perm]
    bq_p = b_q[perm]
    Wgk_p = Wg[0:C][perm]
    wg1k_p = wg1[0:C][perm]
    bvk_p = b_v[perm]

    qwT = chunkT(qw_p)                                 # [128, 2, 256]
    kvwT = chunkT(np.concatenate([Wgk_p, Wg[C:]], 0))  # [128, 2, 512]
    pwT = chunkT(proj_w)
    srwT = np.asarray(sr_w, f32).transpose(1, 2, 3, 0).reshape(2, 128, 4, C)
    srwT = np.ascontiguousarray(srwT.transpose(1, 0, 2, 3)).astype(F16NP)
    aqT = chunkT(a_q)                                  # [128, 2, 8]
    avT = chunkT(Avg)
    bqT = np.ascontiguousarray(bq_p.T.reshape(R, 2, 128)).astype(F16NP)
    bvTk = np.ascontiguousarray(bvk_p.T.reshape(R, 2, 128)).astype(F16NP)

    wbc = np.zeros((128, WBC_COLS), F16NP)
    wbc[:, OC_SRWT:OC_SRWT + 2048] = srwT.reshape(128, 2048)
    wbc[:, OC_ONE1C] = F16NP(1.0 / C)

    blob = np.zeros((128, BLOB_COLS), F16NP)
    blob[:, O_QWT:O_QWT + 512] = qwT.reshape(128, 512)
    blob[:, O_KVWT:O_KVWT + 1024] = kvwT.reshape(128, 1024)
    blob[:, O_PWT:O_PWT + 512] = pwT.reshape(128, 512)
    blob[:, O_AQT:O_AQT + 16] = aqT.reshape(128, 16)
    blob[:, O_AVT:O_AVT + 16] = avT.reshape(128, 16)
    blob[:, O_IDENT:O_IDENT + 128] = np.eye(128, dtype=F16NP)
    blob[0:R, O_BQT:O_BQT + 256] = bqT.reshape(R, 256)
    blob[0:R, O_BVTK:O_BVTK + 256] = bvTk.reshape(R, 256)
    blob[0:R, O_BVTV:O_BVTV + 256] = b_v.T.astype(F16NP)
    blob[0, O_WG1K:O_WG1K + 256] = wg1k_p.reshape(2, 128).astype(F16NP).reshape(256)
    blob[0, O_WG1V:O_WG1V + 256] = wg1[C:].astype(F16NP)
    blob[0, O_AVG1:O_AVG1 + 8] = avg1.astype(F16NP)
    blob[0, O_ONESR:O_ONESR + 128] = F16NP(1.0)
    blob[0, O_PBROW:O_PBROW + 256] = pb_eff.astype(F16NP)

    def pcols(v):  # [n*128] -> [128, n]
        v = np.asarray(v, f32)
        return np.ascontiguousarray(v.reshape(-1, 128).T)

    wf = np.zeros((128, 4), f32)
    wf[:, 0:2] = pcols(qb_p)
    wf[:, 2:4] = pcols(np.asarray(sr_b, f32))
    return dict(wbc=wbc, wb=blob, wf=wf, pb=pb_eff)


def kernel(x, q_w, q_b, kv_w, kv_b, proj_w, proj_b, a_q, b_q, a_v, b_v,
           sr_w, sr_b, ln_g, ln_b, H, W):
    from concourse.bass_utils import run_bass_kernel_spmd

    x = np.asarray(x, np.float32)
    assert x.shape == (B, N, C) and int(H) == 64 and int(W) == 64

    if "nc" not in _CACHE:
        _CACHE["nc"] = _build_program()
    nc = _CACHE["nc"]

    shared = _prep_shared(q_w, q_b, kv_w, kv_b, proj_w, proj_b, a_q, b_q,
                          a_v, b_v, sr_w, sr_b, ln_g, ln_b)
    in_maps = []
    for c in range(NCORES):
        b, j = c // 4, c % 4
        xb = np.roll(x[b], -NB * j, axis=0)             # own block at rows 0:1024
        xT = np.ascontiguousarray(xb.T.astype(F16NP))   # [256, 4096]
        xT = np.ascontiguousarray(
            xT.reshape(2, 128, N).transpose(1, 0, 2))   # [128, 2, 4096]
        in_maps.append(dict(shared, xT=xT))

    res = run_bass_kernel_spmd(nc, in_maps, list(range(NCORES)))
    _CACHE["res"] = res
    out = np.empty((B, N, C), np.float32)
    for c in range(NCORES):
        b, j = c // 4, c % 4
        out[b, NB * j:NB * (j + 1)] = res.results[c]["out"]
    return out


# revision 24
# speedup vs baseline: 1.0916x; 1.0916x over previous
"""Trainium2 Bass kernel for PVT-style spatial-reduction attention (v3).

Shapes (hardcoded): x [2, 4096, 256], HEAD=8, dh=32, SR=2, R=8, H=W=64.
Sharding: core c = (batch b = c//4, query block j = c%4). Each core computes
q/attention/proj for its 1024 query rows and redundantly computes the small
conv+LN+KV path for its batch. Per-core x is pre-rotated on host so each
core's own query block is rows 0:1024.

v3 design:
- fp16 compute chain (conv/kv/q/pv/proj); fp8e4m3 DoubleRow score matmuls
  (0.5 cyc/row) via host-permuted q/k projection columns + per-head repack
  DMAs to 32-aligned lane bases.
- Transposed pv (pts stationary): 33-row pv matmuls, per-partition softmax
  denominators (single reciprocal + stride-0 scale, no broadcasts).
- V computed token-major directly (xsb stationary) - no PE transposes.
- Softmax exp split across ScalarE (true Exp) and DVE (one-op Schraudolph
  int16 bit-trick into fp16); per-head schedule interleaves the engines.
- Attention starts after strip 0: scores for kv blocks 0-3 of all heads are
  emitted interleaved with strip-1 conv/kv so ScalarE/DVE start ~15us in.
- Few large DMAs; conv weights in a separate first blob for an early start.

PSUM (8 banks): P1 2x[128,1024]f32 (4) for q-path/scores/proj, CVKV
2x[128,512]f32 (2) for conv/kv/v/transposes, PV 2x[128,8,33]f32 (2) for pv.
"""
import sys

if "/opt/trn_rl_repo" not in sys.path:
    sys.path.insert(0, "/opt/trn_rl_repo")

import numpy as np

F16NP = np.float16

HEAD, DH, C, N, B, M, R = 8, 32, 256, 4096, 2, 1024, 8
NB = N // 4          # query rows per core
SCALE = DH ** -0.5
NCORES = 8
MAGIC = 0x5F3759DF
LOG2E8 = 1477.3195879  # 2^10 / ln 2
BITB = 15317.95        # tuned fp16 Schraudolph offset (trunc semantics)

# per-head exp engine schedule over mc=0..7 (A=ScalarE exp, D=DVE bit trick);
# alternating 6A2D / 5A3D heads -> 44 ACT / 20 DVE tiles.
EXP_SCHED = ["AADAADAA"] * 5 + ["ADAADAAD"] * 1 + ["ADADADAA"] * 2

# conv blob (wbc): srwT 2048 | ones1c 1
OC_SRWT, OC_ONE1C = 0, 2048
WBC_COLS = 2064
# main blob (wb) column offsets (fp16 elements)
O_QWT, O_KVWT, O_PWT = 0, 512, 1536
O_AQT, O_AVT, O_IDENT, O_BQT = 2048, 2064, 2080, 2208
O_BVTK, O_BVTV, O_WG1K, O_WG1V = 2464, 2720, 2976, 3232
O_AVG1, O_ONESR, O_PBROW = 3488, 3496, 3624
BLOB_COLS = 3880

_CACHE = {}
DEBUG = False


def _build_program():
    import concourse.bass as bass
    import concourse.tile as tile
    from concourse.bacc import Bacc
    from concourse import mybir

    F32 = mybir.dt.float32
    F16 = mybir.dt.float16
    FP8 = mybir.dt.float8e4
    I16 = mybir.dt.int16
    I32 = mybir.dt.int32
    U8 = mybir.dt.uint8
    AF = mybir.ActivationFunctionType
    ALU = mybir.AluOpType
    PM = mybir.MatmulPerfMode

    nc = Bacc()
    P = 128
    ST = 512       # kv tokens per strip

    def s0(ap, n):
        # stride-0 broadcast along a new innermost free dim
        return bass.AP(tensor=ap.tensor, offset=ap.offset,
                       ap=[list(d) for d in ap.ap] + [[0, n]])

    def bcast(ap, nparts):
        return bass.AP(tensor=ap.tensor, offset=ap.offset,
                       ap=[[0, nparts]] + [list(d) for d in ap.ap])

    xT_d = nc.declare_dram_parameter("xT", [P, 2, N], F16, isOutput=False)
    wbc_d = nc.declare_dram_parameter("wbc", [P, WBC_COLS], F16, isOutput=False)
    wb_d = nc.declare_dram_parameter("wb", [P, BLOB_COLS], F16, isOutput=False)
    wf_d = nc.declare_dram_parameter("wf", [P, 4], F32, isOutput=False)
    pb_d = nc.declare_dram_parameter("pb", [C], F32, isOutput=False)
    out_d = nc.declare_dram_parameter("out", [NB, C], F32, isOutput=True)
    if DEBUG:
        dbg = {
            "d_xsb": nc.declare_dram_parameter("d_xsb", [2, P, 2, 512], F16, isOutput=True),
            "d_an": nc.declare_dram_parameter("d_an", [2, P, 4], F32, isOutput=True),
            "d_k8": nc.declare_dram_parameter("d_k8", [P, 2, M], U8, isOutput=True),
            "d_q8": nc.declare_dram_parameter("d_q8", [P, 2, NB], U8, isOutput=True),
            "d_kA": nc.declare_dram_parameter("d_kA", [P, 2, M], U8, isOutput=True),
            "d_qA": nc.declare_dram_parameter("d_qA", [P, 2, NB], U8, isOutput=True),
            "d_vsb": nc.declare_dram_parameter("d_vsb", [2, P, 4, HEAD, DH + 1], F16, isOutput=True),
            "d_pts0": nc.declare_dram_parameter("d_pts0", [P, 8, NB], F16, isOutput=True),
            "d_rec": nc.declare_dram_parameter("d_rec", [HEAD, P, 8], F32, isOutput=True),
            "d_onorm": nc.declare_dram_parameter("d_onorm", [P, 8, HEAD, DH], F16, isOutput=True),
            "d_outT": nc.declare_dram_parameter("d_outT", [P, 2, NB], F16, isOutput=True),
        }

    with tile.TileContext(nc) as tc:
        with tc.tile_pool(name="wgt", bufs=1) as WGT, \
             tc.tile_pool(name="acts", bufs=1) as ACTS, \
             tc.tile_pool(name="str", bufs=2) as STR, \
             tc.tile_pool(name="tmp", bufs=2) as TMP, \
             tc.tile_pool(name="pts", bufs=8) as PTS, \
             tc.tile_pool(name="fin", bufs=8) as FIN, \
             tc.tile_pool(name="p1", bufs=3, space="PSUM") as P1, \
             tc.tile_pool(name="cvkv", bufs=2, space="PSUM") as CVKV, \
             tc.tile_pool(name="dscr", bufs=2, space="DRAM") as DSCR:

            # ---------------- loads (order matters for early start) ----------
            wbc = WGT.tile([P, WBC_COLS], F16, tag="wbc")
            nc.sync.dma_start(out=wbc[:], in_=wbc_d[:])
            wf = WGT.tile([P, 4], F32, tag="wf")
            nc.sync.dma_start(out=wf[:], in_=wf_d[:])
            xs0 = ACTS.tile([P, 2, 2048], F16, tag="xT0")
            nc.sync.dma_start(out=xs0[:], in_=xT_d[:, :, 0:2048])
            wb = WGT.tile([P, BLOB_COLS], F16, tag="wb")
            nc.sync.dma_start(out=wb[:], in_=wb_d[:])
            xs1 = ACTS.tile([P, 2, 2048], F16, tag="xT1")
            nc.sync.dma_start(out=xs1[:], in_=xT_d[:, :, 2048:4096])
            pbB = WGT.tile([P, C], F32, tag="pbB")
            nc.sync.dma_start(out=pbB[:], in_=bcast(pb_d.ap(), P))
            xTs = [xs0, xs1]

            srwT = wbc[:, OC_SRWT:OC_SRWT + 2048].rearrange(
                "p (ch t c) -> p ch t c", ch=2, t=4)
            ones1c = wbc[:, OC_ONE1C:OC_ONE1C + 1]
            qwT = wb[:, O_QWT:O_QWT + 512].rearrange("p (ch c) -> p ch c", ch=2)
            kvwT = wb[:, O_KVWT:O_KVWT + 1024].rearrange("p (ch c) -> p ch c", ch=2)
            pwT = wb[:, O_PWT:O_PWT + 512].rearrange("p (ch c) -> p ch c", ch=2)
            aqT = wb[:, O_AQT:O_AQT + 16].rearrange("p (ch r) -> p ch r", ch=2)
            avT = wb[:, O_AVT:O_AVT + 16].rearrange("p (ch r) -> p ch r", ch=2)
            ident = wb[:, O_IDENT:O_IDENT + 128]
            bqT = wb[0:R, O_BQT:O_BQT + 256].rearrange("r (ch c) -> r ch c", ch=2)
            bvTk = wb[0:R, O_BVTK:O_BVTK + 256].rearrange("r (ch c) -> r ch c", ch=2)
            bvTv = wb[0:R, O_BVTV:O_BVTV + 256]
            wg1k = wb[0:1, O_WG1K:O_WG1K + 256].rearrange("a (ch c) -> a ch c", ch=2)
            wg1v = wb[0:1, O_WG1V:O_WG1V + 256]
            avg1 = wb[0:1, O_AVG1:O_AVG1 + 8]
            onesr = wb[0:1, O_ONESR:O_ONESR + 128]
            pbrow = wb[0:1, O_PBROW:O_PBROW + 256]

            # persistent activations
            k8tmp = ACTS.tile([P, 2, M], FP8, tag="k8tmp")
            q8tmp = ACTS.tile([P, 2, NB], FP8, tag="q8tmp")
            kO = ACTS.tile([P, 2, M], FP8, tag="kO")
            qO = ACTS.tile([P, 2, NB], FP8, tag="qO")
            outT = ACTS.tile([P, 2, NB], F16, tag="outT")
            onorm = ACTS.tile([P, 8, HEAD, DH], F16, tag="onorm")
            tq = ACTS.tile([R, NB], F16, tag="tq")

            xsbs, sqs, negmus, t2s, vsbs, ans, ascls, abits = \
                [], [], [], [], [], [], [], []

            # ---------------- per-strip pieces as emission chunks ------------
            def conv_oc(s, oc):
                xs_t = xTs[s]
                if oc == 0:
                    xsb_s = STR.tile([P, 2, ST], F16, tag="xsb")
                    xsbs.append(xsb_s)
                xsb_s = xsbs[s]
                cps = CVKV.tile([P, ST], F32, tag="cv")
                first = True
                for cc in range(2):
                    xv = xs_t[:, cc, :].rearrange(
                        "p (i a j b) -> p i a j b", i=16, a=2, j=32, b=2)
                    for di in range(2):
                        for dj in range(2):
                            nc.tensor.matmul(
                                cps[:], srwT[:, cc, di * 2 + dj,
                                             oc * P:(oc + 1) * P],
                                xv[:, :, di, :, dj],
                                start=first,
                                stop=(cc == 1 and di == 1 and dj == 1))
                            first = False
                if s == 0:
                    nc.scalar.activation(out=xsb_s[:, oc, :], in_=cps[:],
                                         func=AF.Identity,
                                         bias=wf[:, 2 + oc:3 + oc])
                else:
                    nc.vector.tensor_scalar_add(
                        out=xsb_s[:, oc, :], in0=cps[:], scalar1=wf[:, 2 + oc:3 + oc])
                if DEBUG and oc == 1:
                    nc.sync.dma_start(out=dbg["d_xsb"][s], in_=xsb_s[:])

            def stats(s):
                xsb_s = xsbs[s]
                sq_s = STR.tile([P, 2, ST], F16, tag="sq")
                nc.gpsimd.tensor_mul(out=sq_s[:], in0=xsb_s[:], in1=xsb_s[:])
                sxp = CVKV.tile([1, ST], F32, tag="cv")
                nc.tensor.matmul(sxp[:], ones1c, xsb_s[:, 0, :], start=True, stop=False)
                nc.tensor.matmul(sxp[:], ones1c, xsb_s[:, 1, :], start=False, stop=True)
                negmu = TMP.tile([1, ST], F16, tag="negmu")
                nc.vector.tensor_scalar_mul(out=negmu[:], in0=sxp[:], scalar1=-1.0)
                negmus.append(negmu)
                sxxp = CVKV.tile([1, ST], F32, tag="cv")
                nc.tensor.matmul(sxxp[:], ones1c, sq_s[:, 0, :], start=True, stop=False)
                nc.tensor.matmul(sxxp[:], ones1c, sq_s[:, 1, :], start=False, stop=True)
                ex2_sb = TMP.tile([1, ST], F32, tag="ex2sb")
                nc.vector.tensor_copy(out=ex2_sb[:], in_=sxxp[:])

                # chunk-major repack [1,512] -> [128,4] via DRAM bounce
                nm_d = DSCR.tile([ST], F16, tag=f"nm{s}")
                nc.sync.dma_start(out=nm_d[:], in_=negmu[:])
                ex_d = DSCR.tile([ST], F32, tag=f"ex{s}")
                nc.sync.dma_start(out=ex_d[:], in_=ex2_sb[:])
                mur = TMP.tile([P, 4], F16, tag="mur")
                nc.sync.dma_start(out=mur[:],
                                  in_=nm_d[:].rearrange("(g p) -> p g", p=P))
                ex2r = TMP.tile([P, 4], F32, tag="ex2r")
                nc.sync.dma_start(out=ex2r[:],
                                  in_=ex_d[:].rearrange("(g p) -> p g", p=P))

                # rstd via quake rsqrt (1 newton), [128,4] chain on DVE
                nmu2 = TMP.tile([P, 4], F32, tag="nmu2")
                nc.vector.scalar_tensor_tensor(out=nmu2[:], in0=mur[:], scalar=-1.0,
                                               in1=mur[:], op0=ALU.mult, op1=ALU.mult)
                ve = TMP.tile([P, 4], F32, tag="ve")
                nc.vector.scalar_tensor_tensor(out=ve[:], in0=nmu2[:], scalar=1e-5,
                                               in1=ex2r[:], op0=ALU.add, op1=ALU.add)
                hsh = TMP.tile([P, 4], I32, tag="hsh")
                nc.vector.tensor_scalar(out=hsh[:], in0=ve[:].bitcast(I32), scalar1=1,
                                        scalar2=None, op0=ALU.logical_shift_right)
                nc.vector.tensor_scalar(out=hsh[:], in0=hsh[:], scalar1=-1,
                                        scalar2=MAGIC, op0=ALU.mult, op1=ALU.add)
                y0 = hsh[:].bitcast(F32)
                nt = TMP.tile([P, 4], F32, tag="nt")
                nc.vector.tensor_mul(out=nt[:], in0=y0, in1=y0)
                nc.vector.scalar_tensor_tensor(out=nt[:], in0=nt[:], scalar=-0.5,
                                               in1=ve[:], op0=ALU.mult, op1=ALU.mult)
                nc.vector.tensor_scalar_add(out=nt[:], in0=nt[:], scalar1=1.5)
                an_s = STR.tile([P, 4], F32, tag="an")
                nc.vector.tensor_mul(out=an_s[:], in0=y0, in1=nt[:])
                ascl_s = STR.tile([P, 4], F32, tag="ascl")
                nc.vector.tensor_scalar_mul(out=ascl_s[:], in0=an_s[:], scalar1=SCALE)
                abit_s = STR.tile([P, 4], F32, tag="abit")
                nc.vector.tensor_scalar_mul(out=abit_s[:], in0=ascl_s[:],
                                            scalar1=LOG2E8)
                ans.append(an_s)
                ascls.append(ascl_s)
                abits.append(abit_s)
                if DEBUG:
                    nc.sync.dma_start(out=dbg["d_an"][s], in_=an_s[:])

            def t2_and_k(s):
                xsb_s, negmu = xsbs[s], negmus[s]
                t2p = CVKV.tile([R, ST], F32, tag="cv")
                nc.tensor.matmul(t2p[:], avT[:, 0, :], xsb_s[:, 0, :], start=True, stop=False)
                nc.tensor.matmul(t2p[:], avT[:, 1, :], xsb_s[:, 1, :], start=False, stop=False)
                nc.tensor.matmul(t2p[:], avg1, negmu[:], start=False, stop=True)
                t2 = TMP.tile([R, ST], F16, tag="t2")
                nc.vector.tensor_copy(out=t2[:], in_=t2p[:])
                t2s.append(t2)
                for kvoc in range(2):
                    kps = CVKV.tile([P, ST], F32, tag="cv")
                    nc.tensor.matmul(kps[:], kvwT[:, 0, kvoc * P:(kvoc + 1) * P],
                                     xsb_s[:, 0, :], start=True, stop=False)
                    nc.tensor.matmul(kps[:], kvwT[:, 1, kvoc * P:(kvoc + 1) * P],
                                     xsb_s[:, 1, :], start=False, stop=False)
                    nc.tensor.matmul(kps[:], wg1k[:, kvoc, :], negmu[:],
                                     start=False, stop=False)
                    nc.tensor.matmul(kps[:], bvTk[:, kvoc, :], t2[:],
                                     start=False, stop=True)
                    if s == 0:
                        nc.scalar.copy(out=k8tmp[:, kvoc, s * ST:(s + 1) * ST],
                                       in_=kps[:])
                    else:
                        nc.vector.tensor_copy(
                            out=k8tmp[:, kvoc, s * ST:(s + 1) * ST], in_=kps[:])

            def k_repack(s):
                # odd heads h=2a+1 (packed base 16h=32a+16) -> kO base 32a
                for a in range(4):
                    h = 2 * a + 1
                    nc.gpsimd.dma_start(
                        out=kO[32 * a:32 * a + 16, :, s * ST:(s + 1) * ST],
                        in_=k8tmp[16 * h:16 * h + 16, :, s * ST:(s + 1) * ST])

            def v_path(s):
                # token-major v: out[m, vchan] via xsb/negmu/t2 as stationary
                xsb_s, negmu, t2, an_s = xsbs[s], negmus[s], t2s[s], ans[s]
                vsb_s = STR.tile([P, 4, HEAD, DH + 1], F16, tag="vsb")
                for ml in range(4):
                    sl = slice(ml * P, (ml + 1) * P)
                    vP = CVKV.tile([P, C], F32, tag="cv")
                    nc.tensor.matmul(vP[:], xsb_s[:, 0, sl], kvwT[:, 0, 256:512],
                                     start=True, stop=False)
                    nc.tensor.matmul(vP[:], xsb_s[:, 1, sl], kvwT[:, 1, 256:512],
                                     start=False, stop=False)
                    nc.tensor.matmul(vP[:], negmu[:, sl], wg1v,
                                     start=False, stop=False, tile_position=(0, 0))
                    nc.tensor.matmul(vP[:], t2[:, sl], bvTv,
                                     start=False, stop=True, tile_position=(0, 0))
                    nc.vector.tensor_scalar_mul(
                        out=vsb_s[:, ml, :, 0:DH],
                        in0=vP[:].rearrange("p (h d) -> p h d", d=DH),
                        scalar1=an_s[:, ml:ml + 1])
                nc.gpsimd.memset(vsb_s[:, :, :, DH:DH + 1], 1.0)
                vsbs.append(vsb_s)
                if DEBUG:
                    nc.sync.dma_start(out=dbg["d_vsb"][s], in_=vsb_s[:])

            def q_path():
                xs_t = xTs[0]
                tqp = P1.tile([R, NB], F32, tag="p1")
                for nh in range(2):
                    sl = slice(nh * 512, (nh + 1) * 512)
                    nc.tensor.matmul(tqp[:, sl], aqT[:, 0, :], xs_t[:, 0, sl],
                                     start=True, stop=False)
                    nc.tensor.matmul(tqp[:, sl], aqT[:, 1, :], xs_t[:, 1, sl],
                                     start=False, stop=True)
                nc.vector.tensor_copy(out=tq[:], in_=tqp[:])
                for oc in range(2):
                    qps = P1.tile([P, NB], F32, tag="p1")
                    for nh in range(2):
                        sl = slice(nh * 512, (nh + 1) * 512)
                        nc.tensor.matmul(qps[:, sl],
                                         qwT[:, 0, oc * P:(oc + 1) * P],
                                         xs_t[:, 0, sl], start=True, stop=False)
                        nc.tensor.matmul(qps[:, sl],
                                         qwT[:, 1, oc * P:(oc + 1) * P],
                                         xs_t[:, 1, sl], start=False, stop=False)
                        nc.tensor.matmul(qps[:, sl], bqT[:, oc, :], tq[:, sl],
                                         start=False, stop=True)
                    nc.vector.tensor_scalar_add(
                        out=q8tmp[:, oc, :], in0=qps[:], scalar1=wf[:, oc:oc + 1])
                for a in range(4):
                    h = 2 * a + 1
                    nc.sync.dma_start(
                        out=qO[32 * a:32 * a + 16],
                        in_=q8tmp[16 * h:16 * h + 16])

            # ---------------- attention pieces ----------------
            ptshs = [None] * HEAD

            def score_pair(h, mcp):
                # scores+exp for mc = 2*mcp, 2*mcp+1; even heads read the
                # packed tiles directly (base 16h is 32-aligned), odd heads
                # the repacked kO/qO
                if h % 2 == 0:
                    a, kT, qT8 = h // 2, k8tmp, q8tmp
                else:
                    a, kT, qT8 = h // 2, kO, qO
                if ptshs[h] is None:
                    ptsh = PTS.tile([P, 8, NB], F16, tag="ptsh")
                    ptshs[h] = ptsh
                ptsh = ptshs[h]
                for mc in (2 * mcp, 2 * mcp + 1):
                    s_, ml = mc // 4, mc % 4
                    stile = P1.tile([P, NB], F32, tag="p1")
                    for qc in range(4):
                        nc.tensor.matmul(
                            stile[:, qc * 256:(qc + 1) * 256],
                            kT[32 * a:32 * a + 16, :, mc * P:(mc + 1) * P],
                            qT8[32 * a:32 * a + 16, :, qc * 256:(qc + 1) * 256],
                            start=True, stop=True, perf_mode=PM.DoubleRow,
                            tile_position=(32 * a, 0))
                    if EXP_SCHED[h][mc] == "A":
                        nc.scalar.activation(out=ptsh[:, mc, :], in_=stile[:],
                                             func=AF.Exp,
                                             scale=ascls[s_][:, ml:ml + 1])
                    else:
                        nc.vector.tensor_scalar(
                            out=ptsh[:, mc, :].bitcast(I16), in0=stile[:],
                            scalar1=abits[s_][:, ml:ml + 1], scalar2=BITB,
                            op0=ALU.mult, op1=ALU.add)

            def pv_head(h):
                ptsh = ptshs[h]
                pvq = CVKV.tile([P, 8, DH + 1], F32, tag="cv")
                for qb in range(8):
                    for mc in range(8):
                        nc.tensor.matmul(
                            pvq[:, qb, :],
                            ptsh[:, mc, qb * P:(qb + 1) * P],
                            vsbs[mc // 4][:, mc % 4, h, :],
                            start=(mc == 0), stop=(mc == 7),
                            tile_position=(0, 0))
                rec = TMP.tile([P, 8], F32, tag="rec")
                nc.vector.reciprocal(out=rec[:], in_=pvq[:, :, DH])
                nc.vector.tensor_tensor(out=onorm[:, :, h, :],
                                        in0=pvq[:, :, 0:DH],
                                        in1=s0(rec[:], DH), op=ALU.mult)
                if DEBUG:
                    if h == 0:
                        nc.sync.dma_start(out=dbg["d_pts0"][:], in_=ptsh[:])
                    nc.sync.dma_start(out=dbg["d_rec"][h], in_=rec[:])

            def transpose_ch(ch):
                for qb in range(8):
                    ttile = CVKV.tile([P, P], F16, tag="cv")
                    nc.tensor.transpose(
                        ttile[:],
                        onorm[:, qb, ch * 4:(ch + 1) * 4, :].rearrange(
                            "p h d -> p (h d)"),
                        ident)
                    if ch == 1 and qb % 2 == 0:
                        nc.scalar.copy(out=outT[:, ch, qb * P:(qb + 1) * P],
                                       in_=ttile[:])
                    else:
                        nc.vector.tensor_copy(
                            out=outT[:, ch, qb * P:(qb + 1) * P], in_=ttile[:])

            # ---------------- emission schedule ----------------
            conv_oc(0, 0)
            conv_oc(0, 1)
            stats(0)
            t2_and_k(0)
            k_repack(0)
            q_path()
            v_path(0)

            # interleave strip 1 with early scores (all heads, kv blocks 0-3)
            s1_chunks = [
                lambda: conv_oc(1, 0),
                lambda: conv_oc(1, 1),
                lambda: stats(1),
                lambda: t2_and_k(1),
                lambda: (k_repack(1), v_path(1)),
            ]
            b1_chunks = [(h, mcp) for h in (0, 2, 4, 6, 1, 3, 5, 7)
                         for mcp in (0, 1)]
            bi = 0
            for chunk in s1_chunks:
                chunk()
                for _ in range(2):
                    if bi < len(b1_chunks):
                        score_pair(*b1_chunks[bi])
                        bi += 1
            while bi < len(b1_chunks):
                score_pair(*b1_chunks[bi])
                bi += 1

            # second half: kv blocks 4-7 per head, pv pipelined one head behind
            for h in range(HEAD):
                score_pair(h, 2)
                score_pair(h, 3)
                if h > 0:
                    pv_head(h - 1)
                if h == 4:
                    transpose_ch(0)   # heads 0-3 normalized by now
            pv_head(HEAD - 1)
            if DEBUG:
                nc.sync.dma_start(out=dbg["d_onorm"][:], in_=onorm[:])
            transpose_ch(1)
            if DEBUG:
                nc.sync.dma_start(out=dbg["d_outT"][:], in_=outT[:])

            # ---------------- projection (per-t8, pipelined) ----------------
            for t8 in range(8):
                pp = P1.tile([P, C], F32, tag="p1")
                nc.tensor.matmul(pp[:], outT[:, 0, t8 * P:(t8 + 1) * P],
                                 pwT[:, 0, :], start=True, stop=False)
                nc.tensor.matmul(pp[:], outT[:, 1, t8 * P:(t8 + 1) * P],
                                 pwT[:, 1, :], start=False, stop=False)
                nc.tensor.matmul(pp[:], onesr, pbrow,
                                 start=False, stop=True, tile_position=(0, 0))
                fin = FIN.tile([P, C], F32, tag="fin")
                if t8 % 2 == 0:
                    nc.scalar.copy(out=fin[:], in_=pp[:])
                else:
                    nc.vector.tensor_copy(out=fin[:], in_=pp[:])
                nc.sync.dma_start(out=out_d[t8 * P:(t8 + 1) * P], in_=fin[:])

    nc.finalize()
    return nc


P_ = 128


def _kq_perm():
    # PSUM chunk oc holds half `oc` of all heads: partition p=16h+lane ->
    # original channel 32h + 16*oc + lane
    perm = np.empty(2 * P_, np.int64)
    for oc in range(2):
        for p in range(P_):
            perm[oc * P_ + p] = 32 * (p // 16) + 16 * oc + (p % 16)
    return perm


def _prep_shared(q_w, q_b, kv_w, kv_b, proj_w, proj_b, a_q, b_q, a_v, b_v,
                 sr_w, sr_b, ln_g, ln_b):
    f32 = np.float32

    def chunkT(w):  # [out, in] -> [128, n_in_chunks, out]
        wt = np.ascontiguousarray(np.asarray(w, f32).T)
        ic, oc = wt.shape
        return np.ascontiguousarray(
            wt.reshape(ic // 128, 128, oc).transpose(1, 0, 2)).astype(F16NP)

    kv_w = np.asarray(kv_w, f32)
    a_v = np.asarray(a_v, f32)
    b_v = np.asarray(b_v, f32)
    b_q = np.asarray(b_q, f32)
    q_w = np.asarray(q_w, f32)
    q_b = np.asarray(q_b, f32)
    g = np.asarray(ln_g, f32)
    bb = np.asarray(ln_b, f32)
    proj_w = np.asarray(proj_w, f32)

    # fold LayerNorm gamma into kv/a_v weights; mean via rank-1 correction;
    # k-side constants dropped (softmax shift invariance), v-side constants
    # folded into the projection bias.
    Wg = kv_w * g[None, :]
    wg1 = Wg.sum(1)
    Avg = a_v * g[None, :]
    avg1 = Avg.sum(1)
    wbt = kv_w @ bb + np.asarray(kv_b, f32)
    dconst = b_v @ (a_v @ bb)
    wv_const = wbt[C:] + dconst
    pb_eff = np.asarray(proj_b, f32) + proj_w @ wv_const

    perm = _kq_perm()
    qw_p = q_w[perm]
    qb_p = q_b[perm]
    bq_p = b_q[perm]
    Wgk_p = Wg[0:C][perm]
    wg1k_p = wg1[0:C][perm]
    bvk_p = b_v[perm]

    qwT = chunkT(qw_p)                                 # [128, 2, 256]
    kvwT = chunkT(np.concatenate([Wgk_p, Wg[C:]], 0))  # [128, 2, 512]
    pwT = chunkT(proj_w)
    srwT = np.asarray(sr_w, f32).transpose(1, 2, 3, 0).reshape(2, 128, 4, C)
    srwT = np.ascontiguousarray(srwT.transpose(1, 0, 2, 3)).astype(F16NP)
    aqT = chunkT(a_q)                                  # [128, 2, 8]
    avT = chunkT(Avg)
    bqT = np.ascontiguousarray(bq_p.T.reshape(R, 2, 128)).astype(F16NP)
    bvTk = np.ascontiguousarray(bvk_p.T.reshape(R, 2, 128)).astype(F16NP)

    wbc = np.zeros((128, WBC_COLS), F16NP)
    wbc[:, OC_SRWT:OC_SRWT + 2048] = srwT.reshape(128, 2048)
    wbc[:, OC_ONE1C] = F16NP(1.0 / C)

    blob = np.zeros((128, BLOB_COLS), F16NP)
    blob[:, O_QWT:O_QWT + 512] = qwT.reshape(128, 512)
    blob[:, O_KVWT:O_KVWT + 1024] = kvwT.reshape(128, 1024)
    blob[:, O_PWT:O_PWT + 512] = pwT.reshape(128, 512)
    blob[:, O_AQT:O_AQT + 16] = aqT.reshape(128, 16)
    blob[:, O_AVT:O_AVT + 16] = avT.reshape(128, 16)
    blob[:, O_IDENT:O_IDENT + 128] = np.eye(128, dtype=F16NP)
    blob[0:R, O_BQT:O_BQT + 256] = bqT.reshape(R, 256)
    blob[0:R, O_BVTK:O_BVTK + 256] = bvTk.reshape(R, 256)
    blob[0:R, O_BVTV:O_BVTV + 256] = b_v.T.astype(F16NP)
    blob[0, O_WG1K:O_WG1K + 256] = wg1k_p.reshape(2, 128).astype(F16NP).reshape(256)
    blob[0, O_WG1V:O_WG1V + 256] = wg1[C:].astype(F16NP)
    blob[0, O_AVG1:O_AVG1 + 8] = avg1.astype(F16NP)
    blob[0, O_ONESR:O_ONESR + 128] = F16NP(1.0)
    blob[0, O_PBROW:O_PBROW + 256] = pb_eff.astype(F16NP)

    def pcols(v):  # [n*128] -> [128, n]
        v = np.asarray(v, f32)
        return np.ascontiguousarray(v.reshape(-1, 128).T)

    wf = np.zeros((128, 4), f32)
    wf[:, 0:2] = pcols(qb_p)
    wf[:, 2:4] = pcols(np.asarray(sr_b, f32))
    return dict(wbc=wbc, wb=blob, wf=wf, pb=pb_eff)


def kernel(x, q_w, q_b, kv_w, kv_b, proj_w, proj_b, a_q, b_q, a_v, b_v,
           sr_w, sr_b, ln_g, ln_b, H, W):
    from concourse.bass_utils import run_bass_kernel_spmd

    x = np.asarray(x, np.float32)
    assert x.shape == (B, N, C) and int(H) == 64 and int(W) == 64

    if "nc" not in _CACHE:
        _CACHE["nc"] = _build_program()
    nc = _CACHE["nc"]

    shared = _prep_shared(q_w, q_b, kv_w, kv_b, proj_w, proj_b, a_q, b_q,
                          a_v, b_v, sr_w, sr_b, ln_g, ln_b)
    in_maps = []
    for c in range(NCORES):
        b, j = c // 4, c % 4
        xb = np.roll(x[b], -NB * j, axis=0)             # own block at rows 0:1024
        xT = np.ascontiguousarray(xb.T.astype(F16NP))   # [256, 4096]
        xT = np.ascontiguousarray(
            xT.reshape(2, 128, N).transpose(1, 0, 2))   # [128, 2, 4096]
        in_maps.append(dict(shared, xT=xT))

    res = run_bass_kernel_spmd(nc, in_maps, list(range(NCORES)))
    _CACHE["res"] = res
    out = np.empty((B, N, C), np.float32)
    for c in range(NCORES):
        b, j = c // 4, c % 4
        out[b, NB * j:NB * (j + 1)] = res.results[c]["out"]
    return out
